# revision 15
# baseline (speedup 1.0000x reference)
"""Trainium2 Bass kernel for nn_CPA_Loss (CPA seg loss + dice).

Math (verified vs reference at ~3e-6 rel err):
  gf[j,k] = (c_k/c_j)^0.8 if c_j > c_k else 1   (lower triangle incl diag = 1)
  pf[j]   = 4 / (cos(local_proto_j, global_proto_j) + 3)
  per pixel n with target t, logits l[0..3] (NO max-shift needed; EPS terms
  inside/under logs are provably below tolerance):
    e_k = exp(l_k);  S = sum_k e_k;  d_j = sum_k gf[j,k] e_k   (d_3 = S)
    loss_n = -0.5 * [ (pf_t+1)*l_t - pf_t*log(d_t) - log(S) ]
  dice: prob1 = e_1/S;  per batch b: (2*sum(prob1*t)+1)/(sum prob1 + sum t + 1)
  out = mean_n loss_n + 1 - mean_b dice_b

Sharding: data-parallel over batch, 2 samples per core across 8 cores.
Each core reduces its 131072 pixels to a small [128, 19] partial tensor of
per-class masked sums; the host applies the tiny pf weights and does the
final combine in f64 (pf never needs to touch the device).

Constraint honored throughout: every instruction may carry at most ONE
sync wait on a DMA semaphore (walrus S3D3 limit) — so each compute op
reads tiles written by at most one DMA instruction (or by compute engines,
whose program order collapses to one wait).
"""

import sys

if "/opt/trn_rl_repo" not in sys.path:
    sys.path.insert(0, "/opt/trn_rl_repo")

import numpy as np

import concourse.bass as bass
import concourse.bacc as bacc
import concourse.mybir as mybir
from concourse import tile
from concourse.bass_utils import run_bass_kernel_spmd

F32 = mybir.dt.float32
I32 = mybir.dt.int32
ALU = mybir.AluOpType
ACTF = mybir.ActivationFunctionType

# class-count global factor (compile-time constants from the problem)
_CC = np.array([500000.0, 60000.0, 8000.0, 900.0], dtype=np.float64)
_BETA = 0.8
GF = np.where(_CC[:, None] > _CC[None, :], (_CC[None, :] / _CC[:, None]) ** _BETA, 1.0)
G01, G02, G03 = float(GF[0, 1]), float(GF[0, 2]), float(GF[0, 3])
G12, G13 = float(GF[1, 2]), float(GF[1, 3])
G23 = float(GF[2, 3])

N_CORES = 8
B_LOC = 2  # batch samples per core
TTR_MODE = "fallback"  # "isa" = fused tensor_tensor_reduce; "fallback" = mul+reduce

# res columns (grouped by producing engine so each output DMA waits on one
# engine only):
#   DVE block: 0-7  A_{j,b} = sum m_j*l_j   (col = 2*j+b)
#              8-11 B_j     = sum m_j*log d_j
#              12-13 pt_b   = sum prob1*t
#              14-15 tsum_b = sum t
#   ACT block: 16 C = sum log S ; 17-18 psum_b = sum prob1
RES_COLS = 19


def build_nc(W: int = 512):
    """Build the per-core Bass module. W = free width per (batch, partition)
    so each batch sample is 128*W pixels (W=512 for the real 256x256 plane)."""
    nc = bacc.Bacc(None)
    lg = nc.dram_tensor("lg", [B_LOC, 4, 128, W], F32, kind="ExternalInput")
    tg = nc.dram_tensor("tg", [B_LOC, 128, W], I32, kind="ExternalInput")
    res_d = nc.dram_tensor("res", [128, RES_COLS], F32, kind="ExternalOutput")

    with tile.TileContext(nc) as tc:
        with tc.tile_pool(name="main", bufs=1) as pool:
            L = pool.tile([128, 4, B_LOC, W], F32)  # logits, channel slabs
            E = pool.tile([128, 4, B_LOC, W], F32)  # exp(l)
            D = pool.tile([128, 4, B_LOC, W], F32)  # d0,d1,d2,S
            LD = pool.tile([128, 3, B_LOC, W], F32)  # log d0..d2
            LS = pool.tile([128, B_LOC, W], F32)  # log S
            TI = pool.tile([128, B_LOC, W], I32)
            TF = pool.tile([128, B_LOC, W], F32)
            M = pool.tile([128, 4, B_LOC, W], F32)  # one-hot masks
            P1 = pool.tile([128, B_LOC, W], F32)
            P2 = pool.tile([128, B_LOC, W], F32)
            Q = pool.tile([128, B_LOC, W], F32)
            PR = pool.tile([128, B_LOC, W], F32)  # prob1
            SC = pool.tile([128, B_LOC, W], F32)  # junk out for TTRs
            RES = pool.tile([128, RES_COLS], F32)

            # ---- loads ----
            for b in range(B_LOC):
                nc.sync.dma_start(
                    out=L[:, :, b], in_=lg[b].rearrange("k p w -> p k w")
                )
            nc.sync.dma_start(out=TI[:], in_=tg.rearrange("b p w -> p b w"))

            # ---- elementwise ----
            # e = exp(l); per-b so each ACT op waits on exactly one DMA
            for b in range(B_LOC):
                nc.scalar.activation(E[:, :, b], L[:, :, b], ACTF.Exp)
            # t as f32
            nc.vector.tensor_copy(TF[:], TI[:])
            # masks m_j = (t == j)
            for j in range(4):
                nc.vector.tensor_scalar(M[:, j], TF[:], float(j), None, ALU.is_equal)
            # prefix sums and d_j
            nc.vector.tensor_add(P1[:], E[:, 0], E[:, 1])
            nc.vector.tensor_add(P2[:], P1[:], E[:, 2])
            nc.vector.tensor_add(D[:, 3], P2[:], E[:, 3])  # S
            nc.vector.scalar_tensor_tensor(
                D[:, 2], E[:, 3], G23, P2[:], ALU.mult, ALU.add
            )
            nc.vector.scalar_tensor_tensor(
                D[:, 1], E[:, 3], G13, P1[:], ALU.mult, ALU.add
            )
            nc.vector.scalar_tensor_tensor(
                D[:, 1], E[:, 2], G12, D[:, 1], ALU.mult, ALU.add
            )
            nc.vector.scalar_tensor_tensor(
                D[:, 0], E[:, 1], G01, E[:, 0], ALU.mult, ALU.add
            )
            nc.vector.scalar_tensor_tensor(
                D[:, 0], E[:, 2], G02, D[:, 0], ALU.mult, ALU.add
            )
            nc.vector.scalar_tensor_tensor(
                D[:, 0], E[:, 3], G03, D[:, 0], ALU.mult, ALU.add
            )
            # logs: three d-slabs in one pass; logS separately with fused sum C
            nc.scalar.activation(LD[:], D[:, 0:3], ACTF.Ln)
            nc.scalar.activation(LS[:], D[:, 3], ACTF.Ln, accum_out=RES[:, 16:17])

            def ttr(acc, in0, in1, sc_out):
                if TTR_MODE == "isa":
                    nc.vector.tensor_tensor_reduce(
                        out=sc_out,
                        in0=in0,
                        in1=in1,
                        scale=1.0,
                        scalar=0.0,
                        op0=ALU.mult,
                        op1=ALU.add,
                        accum_out=acc,
                    )
                else:
                    nc.vector.tensor_mul(sc_out, in0, in1)
                    nc.vector.tensor_reduce(acc, sc_out, mybir.AxisListType.X, ALU.add)

            # A_{j,b} = sum m_j*l_j  (per-b: L is written by per-b DMAs)
            for j in range(4):
                for b in range(B_LOC):
                    ttr(
                        RES[:, 2 * j + b : 2 * j + b + 1],
                        L[:, j, b],
                        M[:, j, b],
                        SC[:, b],
                    )
            # B_j = sum m_j * log d_j  (inputs all compute-written)
            flat = lambda ap: ap.rearrange("p b w -> p (b w)")
            for j in range(3):
                ttr(RES[:, 8 + j : 9 + j], flat(LD[:, j]), flat(M[:, j]), flat(SC[:]))
            ttr(RES[:, 11:12], flat(LS[:]), flat(M[:, 3]), flat(SC[:]))

            # dice: prob1 = exp(l1 - logS); per-b (L DMA wait constraint)
            for b in range(B_LOC):
                nc.vector.tensor_sub(Q[:, b], L[:, 1, b], LS[:, b])
                nc.scalar.activation(
                    PR[:, b], Q[:, b], ACTF.Exp, accum_out=RES[:, 17 + b : 18 + b]
                )
                ttr(RES[:, 12 + b : 13 + b], PR[:, b], TF[:, b], SC[:, b])
            nc.vector.tensor_reduce(RES[:, 14:16], TF[:], mybir.AxisListType.X, ALU.add)

            # ---- stores: one DMA per producing engine ----
            nc.sync.dma_start(out=res_d[:, 0:16], in_=RES[:, 0:16])
            nc.sync.dma_start(out=res_d[:, 16:19], in_=RES[:, 16:19])
    nc.finalize()
    return nc


def _host_pf(local_proto: np.ndarray, global_proto: np.ndarray) -> np.ndarray:
    lp = local_proto.astype(np.float32)
    gp = global_proto.astype(np.float32)
    eps = np.float32(1e-6)
    cos = (lp * gp).sum(-1) / (
        np.linalg.norm(lp, axis=-1) * np.linalg.norm(gp, axis=-1) + eps
    )
    return np.float32(4.0) / (cos + np.float32(3.0))  # (1+TAU)/(cos+TAU)


def _shard_inputs(logits, targets, W):
    in_maps = []
    for c in range(N_CORES):
        lg = np.ascontiguousarray(
            logits[c * B_LOC : (c + 1) * B_LOC].reshape(B_LOC, 4, 128, W)
        )
        tg = np.ascontiguousarray(
            targets[c * B_LOC : (c + 1) * B_LOC].reshape(B_LOC, 128, W)
        )
        in_maps.append({"lg": lg, "tg": tg})
    return in_maps


def _combine(res_list, pf):
    """res_list: 8 arrays [128, RES_COLS]; pf: [4] -> (loss_sum, dice_sum)."""
    pf = pf.astype(np.float64)
    loss_sum = 0.0
    dice_sum = 0.0
    for r in res_list:
        r = r.astype(np.float64)
        a = sum(
            (pf[j] + 1.0) * (r[:, 2 * j].sum() + r[:, 2 * j + 1].sum())
            for j in range(4)
        )
        bsum = sum(pf[j] * r[:, 8 + j].sum() for j in range(4))
        c = r[:, 16].sum()
        loss_sum += -0.5 * (a - bsum - c)
        for b in range(B_LOC):
            pt = r[:, 12 + b].sum()
            ts = r[:, 14 + b].sum()
            psum = r[:, 17 + b].sum()
            dice_sum += (2.0 * pt + 1.0) / (psum + ts + 1.0)
    return loss_sum, dice_sum


_NC_CACHE = {}


def kernel(logits, targets, local_proto, global_proto, _trace=False):
    logits = np.asarray(logits, dtype=np.float32)
    targets = np.asarray(targets, dtype=np.int32)
    W = 512
    n_pix = 16 * 256 * 256

    if W not in _NC_CACHE:
        _NC_CACHE[W] = build_nc(W)
    nc = _NC_CACHE[W]

    pf = _host_pf(np.asarray(local_proto), np.asarray(global_proto))
    in_maps = _shard_inputs(logits, targets, W)

    kres = run_bass_kernel_spmd(nc, in_maps, list(range(N_CORES)), trace=_trace)
    res_list = [kres.results[c]["res"] for c in range(N_CORES)]

    loss_sum, dice_sum = _combine(res_list, pf)
    loss_mean = loss_sum / n_pix
    dice_loss = 1.0 - dice_sum / 16.0
    out = np.float32(loss_mean + dice_loss)
    if _trace:
        return out, kres
    return out
